# revision 17
# baseline (speedup 1.0000x reference)
"""Causal self-attention (B=2, T=4096, C=768, H=12, Dh=64) on 8 TRN2 NeuronCores.

Sharding: batch x head-groups. Core c handles batch b = c//4 and the 3 heads
hh = 3*(c%4) .. hh+2 of that batch (data parallel on B, tensor parallel on
heads for the qkv / out projections). Each core computes a partial output
y_c = attn_out(heads) @ W_out[head rows]; the host sums the 4 partials per
batch and adds b_out.

Device-side layout (per core, identical SPMD program):
  xt    [768, 4096]  x[b].T (host pre-transposed so C lands on partitions)
  wqkv  [768, 576]   columns permuted to [q0 q1 | k0 k1 | q2 k2 | v0 v1 v2]
  bqkv  [576]        same permutation
  wout  [192, 768]   rows for this core's heads
  y     [4096, 768]  partial output (no b_out)

Projection phase produces:
  A  = [q0|q1]^T  [128, T]   (head0 on partitions 0-63, head1 on 64-127)
  B_ = [k0|k1]^T  [128, T]
  Cc = [q2|k2]^T  [128, T]
  v_st [128, 32, 3, 65]      v in [token, d] layout per 128-token block,
                             col 64 = 1.0 (gives softmax row-sums for free)

Attention per (q-super of 1024, head): s^T tiles [128 k, 1024 q] via
matmul(lhsT=k_chunk, rhs=q_super), additive causal mask on the diagonal
128-block, exp on ACT (scale=1/8, no max subtraction: logits are ~N(0,1) so
exp never overflows), then av^T [65, 1024] accumulates matmul(lhsT=v_aug,
rhs=p^T) over k-blocks. Row 64 of av^T is the softmax denominator; normalize
with reciprocal_approx_fast + gpsimd partition-broadcast + DVE multiply into
attnT [64, 3, T]. Out-projection contracts attnT (3x K=64 matmuls) with wout.

All matmuls run in float32r (~1.5e-4 rel err, 4x faster than fp32).
"""

import math

import numpy as np

import concourse.bass as bass
import concourse.tile as tile
from concourse import bacc, mybir
from concourse.bass_utils import run_bass_kernel_spmd

F32 = mybir.dt.float32
F32R = mybir.dt.float32r

T = 4096
C = 768
H = 12
DH = 64
HPC = 3  # heads per core
NCORES = 8
SUP = 1024  # q-super width
NSUP = T // SUP
KB = 128  # k-block
NKB = T // KB
SCALE = 1.0 / math.sqrt(DH)

TRACE = False
LAST_RESULT = None
_PROG = None


def build_program(debug=False):
    nc = bacc.Bacc("TRN2", target_bir_lowering=False, debug=False)
    xt_d = nc.dram_tensor("xt", [C, T], F32, kind="ExternalInput").ap()
    wqkv_d = nc.dram_tensor("wqkv", [C, 576], F32, kind="ExternalInput").ap()
    bqkv_d = nc.dram_tensor("bqkv", [576], F32, kind="ExternalInput").ap()
    wout_d = nc.dram_tensor("wout", [192, C], F32, kind="ExternalInput").ap()
    y_d = nc.dram_tensor("y", [T, C], F32, kind="ExternalOutput").ap()
    if debug:
        dbg = {
            name: nc.dram_tensor(name, shape, F32, kind="ExternalOutput").ap()
            for name, shape in [
                ("dbg_A", [128, T]),
                ("dbg_B", [128, T]),
                ("dbg_C", [128, T]),
                ("dbg_D", [128, T]),
                ("dbg_v", [128, NKB * HPC * (DH + 1)]),
                ("dbg_at0", [64, HPC * T]),
                ("dbg_st", [128, SUP]),
                ("dbg_pt", [128, SUP]),
                ("dbg_av", [65, SUP]),
                ("dbg_rec", [1, SUP]),
            ]
        }

    with tile.TileContext(nc) as tc:
        with tc.tile_pool(name="res", bufs=1) as res:
            A = res.tile([128, T], F32R, tag="A")
            B_ = res.tile([128, T], F32R, tag="B")
            Cc = res.tile([128, T], F32R, tag="Cc")
            D = res.tile([128, T], F32R, tag="D")  # [64:128] = copy of q2
            v_st = res.tile([128, NKB, HPC, DH + 1], F32R, tag="v_st")
            wo_sb = res.tile([64, HPC, C], F32R, tag="wo")
            maskadd = res.tile([128, KB], F32, tag="mask")

            for hc in range(HPC):
                nc.sync.dma_start(
                    wo_sb[:, hc, :],
                    wout_d[hc * 64 : (hc + 1) * 64, :].bitcast(F32R),
                )

            # additive causal mask for a diagonal [k=128, q=128] block:
            # keep (0.0) where q >= k, else -1e5 (-> exp == 0)
            nc.gpsimd.memset(maskadd[:], 0.0)
            nc.gpsimd.affine_select(
                out=maskadd[:],
                in_=maskadd[:],
                compare_op=mybir.AluOpType.is_ge,
                fill=-1e5,
                base=0,
                pattern=[[1, KB]],
                channel_multiplier=-1,
            )
            nc.vector.memset(v_st[:, :, :, DH : DH + 1].bitcast(F32), 1.0)

            # ---------------- Phase 1: projections ----------------
            with (
                tc.tile_pool(name="p1", bufs=1) as p1,
                tc.tile_pool(name="xts", bufs=2) as xpool,
                tc.tile_pool(name="pps", bufs=2, space="PSUM") as pps,
                tc.tile_pool(name="vps", bufs=2, space="PSUM") as vps,
            ):
                wq_sb = p1.tile([128, 6, 576], F32R, tag="wq")
                bias_qk = p1.tile([128, 3], F32, tag="bqk")
                bias_v = p1.tile([128, 192], F32, tag="bv")
                bias_v_row = p1.tile([1, 192], F32, tag="bvr")

                for ci in range(6):
                    nc.sync.dma_start(
                        wq_sb[:, ci, :],
                        wqkv_d[ci * 128 : (ci + 1) * 128, :].bitcast(F32R),
                    )
                for m in range(3):
                    nc.sync.dma_start(
                        bias_qk[:, m : m + 1],
                        bqkv_d[m * 128 : (m + 1) * 128].rearrange("(p b) -> p b", b=1),
                    )
                nc.sync.dma_start(
                    bias_v_row[0:1, :],
                    bqkv_d[384:576].rearrange("(b f) -> b f", b=1),
                )
                nc.gpsimd.partition_broadcast(bias_v[:], bias_v_row[0:1, :])

                qk_dest = [A, B_, Cc]
                for ts in range(T // 512):
                    xts = xpool.tile([128, 6, 512], F32R, tag="xts")
                    nc.sync.dma_start(
                        xts[:],
                        xt_d[:, ts * 512 : (ts + 1) * 512]
                        .rearrange("(ci p) n -> p ci n", p=128)
                        .bitcast(F32R),
                    )
                    col0 = ts * 512
                    # q/k rows (transposed layout): psum [qkv-rows, tokens]
                    for m in range(3):
                        psq = pps.tile([128, 512], F32, tag="psq")
                        for ci in range(6):
                            nc.tensor.matmul(
                                psq[:],
                                wq_sb[:, ci, m * 128 : (m + 1) * 128],
                                xts[:, ci, :],
                                start=(ci == 0),
                                stop=(ci == 5),
                            )
                        nc.vector.tensor_scalar_add(
                            out=qk_dest[m][:, col0 : col0 + 512],
                            in0=psq[:],
                            scalar1=bias_qk[:, m : m + 1],
                        )
                    # v in [token, d] layout: psum [tokens, 3*64]
                    for tb in range(4):
                        psv = vps.tile([128, 192], F32, tag="psv")
                        for ci in range(6):
                            nc.tensor.matmul(
                                psv[:],
                                xts[:, ci, tb * 128 : (tb + 1) * 128],
                                wq_sb[:, ci, 384:576],
                                start=(ci == 0),
                                stop=(ci == 5),
                            )
                        kb = ts * 4 + tb
                        nc.vector.tensor_tensor(
                            out=v_st[:, kb, :, 0:DH],
                            in0=psv[:].rearrange("p (h d) -> p h d", h=HPC),
                            in1=bias_v[:].rearrange("p (h d) -> p h d", h=HPC),
                            op=mybir.AluOpType.add,
                        )

            # q2 lives at partitions 0-63 of Cc but k2 at 64-127; matmul needs
            # equal base partitions, so mirror q2 into the upper half of D.
            nc.sync.dma_start(D[64:128, :], Cc[0:64, :])

            if debug:
                nc.sync.dma_start(dbg["dbg_A"], A[:].bitcast(F32))
                nc.sync.dma_start(dbg["dbg_B"], B_[:].bitcast(F32))
                nc.sync.dma_start(dbg["dbg_C"], Cc[:].bitcast(F32))
                nc.sync.dma_start(dbg["dbg_D"], D[:].bitcast(F32))
                nc.sync.dma_start(
                    dbg["dbg_v"],
                    v_st[:].rearrange("p a b c -> p (a b c)").bitcast(F32),
                )

            # ---------------- Phase 2: attention + out-projection ----------------
            with (
                tc.tile_pool(name="p2", bufs=1) as p2,
                tc.tile_pool(name="stps", bufs=2, space="PSUM") as stps,
                tc.tile_pool(name="avps", bufs=2, space="PSUM") as avps,
                tc.tile_pool(name="ptp", bufs=3) as ptp,
                tc.tile_pool(name="nrm", bufs=1) as nrm,
                tc.tile_pool(name="ysb", bufs=2) as ypool,
            ):
                at0 = p2.tile([64, HPC, T], F32R, tag="at0")

                def q_ap(h):
                    return (A[0:64, :], A[64:128, :], D[64:128, :])[h]

                def k_ap(h):
                    return (B_[0:64, :], B_[64:128, :], Cc[64:128, :])[h]

                for qs in range(NSUP):
                    q0 = qs * SUP
                    nkb = (qs + 1) * (SUP // KB)
                    for h in range(HPC):
                        av = avps.tile([65, SUP], F32, tag="av")
                        last_r0 = qs * 8 + 512 // KB - 1  # last kb touching cols [0,512)
                        for kb in range(nkb):
                            t = kb - qs * (SUP // KB)  # >= 0 on the diagonal
                            ext0 = max(t, 0) * KB
                            st = stps.tile([128, SUP], F32, tag="st")
                            c = ext0
                            while c < SUP:
                                ce = min((c // 512 + 1) * 512, SUP)
                                nc.tensor.matmul(
                                    st[:, c:ce],
                                    k_ap(h)[:, kb * KB : (kb + 1) * KB],
                                    q_ap(h)[:, q0 + c : q0 + ce],
                                    start=True,
                                    stop=True,
                                )
                                c = ce
                            if t >= 0:
                                nc.vector.tensor_tensor(
                                    out=st[:, ext0 : ext0 + KB],
                                    in0=st[:, ext0 : ext0 + KB],
                                    in1=maskadd[:],
                                    op=mybir.AluOpType.add,
                                )
                            pt = ptp.tile([128, SUP], F32R, tag="pt")
                            nc.scalar.activation(
                                out=pt[:, ext0:SUP],
                                in_=st[:, ext0:SUP],
                                func=mybir.ActivationFunctionType.Exp,
                                bias=0.0,
                                scale=SCALE,
                            )
                            c = ext0
                            while c < SUP:
                                ce = min((c // 512 + 1) * 512, SUP)
                                stop_kb = last_r0 if ce <= 512 else nkb - 1
                                nc.tensor.matmul(
                                    av[:, c:ce],
                                    v_st[:, kb, h, :],
                                    pt[:, c:ce],
                                    start=(kb == 0),
                                    stop=(kb == stop_kb),
                                )
                                c = ce
                            if debug and qs == 1 and h == 0 and kb == 0:
                                stg = ptp.tile([128, SUP], F32, tag="dbgstage")
                                nc.vector.tensor_copy(stg[:], st[:])
                                nc.sync.dma_start(dbg["dbg_st"], stg[:])
                                nc.sync.dma_start(dbg["dbg_pt"], pt[:].bitcast(F32))
                        # normalize: rows 0-63 / row 64. reciprocal_approx_fast
                        # misreads PSUM at partition offset 64, so evacuate av
                        # to SBUF and DMA-shift the denominator row to
                        # partition 0 first.
                        stg = nrm.tile([65, SUP], F32, tag="avstage")
                        nc.vector.tensor_copy(stg[:], av[:])
                        if debug and qs == 1 and h == 0:
                            nc.sync.dma_start(dbg["dbg_av"], stg[:])
                        l0 = nrm.tile([1, SUP], F32, tag="l0")
                        nc.sync.dma_start(l0[0:1, :], stg[64:65, :])
                        rec = nrm.tile([1, SUP], F32, tag="rec")
                        nc.vector.reciprocal_approx_fast(out=rec[0:1, :], in_=l0[0:1, :])
                        recb = nrm.tile([64, SUP], F32, tag="recb")
                        nc.gpsimd.partition_broadcast(recb[:], rec[0:1, :])
                        nc.vector.tensor_tensor(
                            out=at0[:, h, q0 : q0 + SUP],
                            in0=stg[0:64, :],
                            in1=recb[:],
                            op=mybir.AluOpType.mult,
                        )
                        if debug and qs == 1 and h == 0:
                            nc.sync.dma_start(dbg["dbg_rec"], rec[0:1, :])
                    # out-projection for this q-super's token blocks
                    wo_h = tuple(wo_sb[:, hc, :] for hc in range(HPC))
                    for tb in range(SUP // 128):
                        tcol = q0 + tb * 128
                        yps = stps.tile([128, SUP], F32, tag="st")
                        for rs, re in ((0, 512), (512, C)):
                            for hc in range(HPC):
                                nc.tensor.matmul(
                                    yps[:, rs:re],
                                    at0[:, hc, tcol : tcol + 128],
                                    wo_h[hc][:, rs:re],
                                    start=(hc == 0),
                                    stop=(hc == HPC - 1),
                                )
                        y_sb = ypool.tile([128, C], F32, tag="ysb")
                        nc.vector.tensor_copy(y_sb[:], yps[:, 0:C])
                        nc.sync.dma_start(y_d[tcol : tcol + 128, :], y_sb[:])
                if debug:
                    nc.sync.dma_start(
                        dbg["dbg_at0"],
                        at0[:].rearrange("p a b -> p (a b)").bitcast(F32),
                    )

    nc.compile()
    return nc


def shard_inputs(x, W_qkv, b_qkv, W_out, b_out):
    """Build the per-core input maps (host-side sharding)."""
    x = np.asarray(x, dtype=np.float32)
    W_qkv = np.asarray(W_qkv, dtype=np.float32)
    b_qkv = np.asarray(b_qkv, dtype=np.float32)
    W_out = np.asarray(W_out, dtype=np.float32)
    in_maps = []
    for c in range(NCORES):
        b = c // 4
        hh = (c % 4) * HPC
        h0, h1, h2 = hh, hh + 1, hh + 2

        def qcols(h):
            return list(range(h * DH, (h + 1) * DH))

        def kcols(h):
            return list(range(C + h * DH, C + (h + 1) * DH))

        def vcols(h):
            return list(range(2 * C + h * DH, 2 * C + (h + 1) * DH))

        perm = (
            qcols(h0) + qcols(h1) + kcols(h0) + kcols(h1) + qcols(h2) + kcols(h2)
            + vcols(h0) + vcols(h1) + vcols(h2)
        )
        in_maps.append(
            {
                "xt": np.ascontiguousarray(x[b].T),
                "wqkv": np.ascontiguousarray(W_qkv[:, perm]),
                "bqkv": np.ascontiguousarray(b_qkv[perm]),
                "wout": np.ascontiguousarray(W_out[hh * DH : (hh + HPC) * DH, :]),
            }
        )
    return in_maps


def kernel(x, W_qkv, b_qkv, W_out, b_out):
    global _PROG, LAST_RESULT
    if _PROG is None:
        _PROG = build_program()
    nc = _PROG
    in_maps = shard_inputs(x, W_qkv, b_qkv, W_out, b_out)
    res = run_bass_kernel_spmd(nc, in_maps, list(range(NCORES)), trace=TRACE)
    LAST_RESULT = res
    b_out = np.asarray(b_out, dtype=np.float32)
    y = np.zeros((2, T, C), dtype=np.float32)
    for c in range(NCORES):
        y[c // 4] += res.results[c]["y"]
    y += b_out[None, None, :]
    return y


# revision 40
# speedup vs baseline: 8.6033x; 8.6033x over previous
"""Causal self-attention (B=2, T=4096, C=768, H=12, Dh=64) on 8 TRN2 NeuronCores.

Sharding: batch x head-groups. Core c handles batch b = c//4 and the 3 heads
hh = 3*(c%4) .. hh+2 of that batch (data parallel on B, tensor parallel on
heads for the qkv / out projections). Each core computes a partial output
y_c = attn_out(heads) @ W_out[head rows]; the host sums the 4 partials per
batch and adds b_out.

Device-side layout (per core, identical SPMD program):
  xt    [768, 4096]  x[b].T (host pre-transposed so C lands on partitions)
  wqkv  [768, 576]   columns permuted to [q0 q1 | k0 k1 | q2 k2 | v0 v1 v2]
  bqkv  [576]        same permutation
  wout  [192, 768]   rows for this core's heads
  y     [4096, 768]  partial output (no b_out)

Projection phase produces:
  A  = [q0|q1]^T  [128, T]   (head0 on partitions 0-63, head1 on 64-127)
  B_ = [k0|k1]^T  [128, T]
  Cc = [q2|k2]^T  [128, T]
  v_st [128, 32, 3, 65]      v in [token, d] layout per 128-token block,
                             col 64 = 1.0 (gives softmax row-sums for free)

Attention per (q-super of 1024, head): s^T tiles [128 k, 1024 q] via
matmul(lhsT=k_chunk, rhs=q_super), additive causal mask on the diagonal
128-block, exp on ACT (scale=1/8, no max subtraction: logits are ~N(0,1) so
exp never overflows), then av^T [65, 1024] accumulates matmul(lhsT=v_aug,
rhs=p^T) over k-blocks. Row 64 of av^T is the softmax denominator; normalize
with reciprocal_approx_fast + gpsimd partition-broadcast + DVE multiply into
attnT [64, 3, T]. Out-projection contracts attnT (3x K=64 matmuls) with wout.

All matmuls run in float32r (~1.5e-4 rel err, 4x faster than fp32).
"""

import math

import numpy as np

import concourse.bass as bass
import concourse.tile as tile
from concourse import bacc, mybir
from concourse.bass_utils import run_bass_kernel_spmd

F32 = mybir.dt.float32
F32R = mybir.dt.float32r
I32 = mybir.dt.int32

# Schraudolph exp constants (int32 domain): exp(s*SCALE) ~= bitcast(int32(As*s + Bs))
LOG2E = 1.4426950408889634
SCH_A = 8388608.0 * LOG2E  # * SCALE applied at use site
SCH_B = 8388608.0 * (127.0 - 0.04367744890362246)

T = 4096
C = 768
H = 12
DH = 64
HPC = 3  # heads per core
NCORES = 8
SUP = 1024  # q-super width
NSUP = T // SUP
KB = 128  # k-block
NKB = T // KB
SCALE = 1.0 / math.sqrt(DH)

TRACE = False
LAST_RESULT = None
_PROG = None


DEFAULT_CFG = {
    "st_chunk": 1024,  # st psum tile + exp granularity (512 or 1024)
    "st_bufs": 2,
    "av_bufs": 2,
    "pt_bufs": 3,
    "yevac": "act",  # 'act' | 'dve'
    "nrm_bufs": 1,
    "repeat": 1,  # hardware For_i repetitions of the whole body (benchmarking)
}


def build_program(debug=False, cfg=None):
    cfg = {**DEFAULT_CFG, **(cfg or {})}
    nc = bacc.Bacc("TRN2", target_bir_lowering=False, debug=False)
    xt_d = nc.dram_tensor("xt", [C, T], F32, kind="ExternalInput").ap()
    wqkv_d = nc.dram_tensor("wqkv", [C, 576], F32, kind="ExternalInput").ap()
    bqkv_d = nc.dram_tensor("bqkv", [576], F32, kind="ExternalInput").ap()
    wout_d = nc.dram_tensor("wout", [192, C], F32, kind="ExternalInput").ap()
    y_d = nc.dram_tensor("y", [T, C], F32, kind="ExternalOutput").ap()
    if debug:
        dbg = {
            name: nc.dram_tensor(name, shape, F32, kind="ExternalOutput").ap()
            for name, shape in [
                ("dbg_A", [128, T]),
                ("dbg_B", [128, T]),
                ("dbg_C", [128, T]),
                ("dbg_D", [128, T]),
                ("dbg_v", [128, NKB * HPC * (DH + 1)]),
                ("dbg_at0", [64, HPC * T]),
                ("dbg_st", [128, SUP]),
                ("dbg_pt", [128, SUP]),
                ("dbg_av", [65, SUP]),
                ("dbg_rec", [1, SUP]),
            ]
        }

    with tile.TileContext(nc) as tc:
        with tc.tile_pool(name="res", bufs=1) as res:
            A = res.tile([128, T], F32R, tag="A")
            B_ = res.tile([128, T], F32R, tag="B")
            Cc = res.tile([128, T], F32R, tag="Cc")
            D = res.tile([128, T], F32R, tag="D")  # [64:128] = copy of q2
            v_st = res.tile([128, NKB, HPC, DH + 1], F32R, tag="v_st")
            wo01 = res.tile([128, C], F32R, tag="wo01")
            wo2 = res.tile([64, C], F32R, tag="wo2")
            mask01 = res.tile([128, SUP], F32, tag="mask")

            nc.sync.dma_start(wo01[:], wout_d[0:128, :].bitcast(F32R))
            nc.sync.dma_start(wo2[:], wout_d[128:192, :].bitcast(F32R))

            # multiplicative causal mask: cols 0-127 hold the diagonal-block
            # triangle (1.0 where q >= k else 0.0), cols 128.. are all 1.0
            nc.gpsimd.memset(mask01[:], 1.0)
            nc.gpsimd.affine_select(
                out=mask01[:, 0:KB],
                in_=mask01[:, 0:KB],
                compare_op=mybir.AluOpType.is_ge,
                fill=0.0,
                base=0,
                pattern=[[1, KB]],
                channel_multiplier=-1,
            )
            # additive variant (0 / -1e5) for diagonal tiles kept on ACT
            maskadd = res.tile([128, KB], F32, tag="maskadd")
            nc.gpsimd.memset(maskadd[:], 0.0)
            nc.gpsimd.affine_select(
                out=maskadd[:],
                in_=maskadd[:],
                compare_op=mybir.AluOpType.is_ge,
                fill=-1e5,
                base=0,
                pattern=[[1, KB]],
                channel_multiplier=-1,
            )
            nc.vector.memset(v_st[:, :, :, DH : DH + 1].bitcast(F32), 1.0)

            import contextlib

            rep_ctx = (
                tc.For_i(0, cfg["repeat"], 1)
                if cfg.get("repeat", 1) > 1
                else contextlib.nullcontext()
            )
            rep_ctx.__enter__()

            # ---------------- Phase 1: projections ----------------
            with (
                tc.tile_pool(name="p1", bufs=1) as p1,
                tc.tile_pool(name="xts", bufs=2) as xpool,
                tc.tile_pool(name="pps", bufs=2, space="PSUM") as pps,
                tc.tile_pool(name="vps", bufs=2, space="PSUM") as vps,
            ):
                # 640 = 576 + 64 zero pad so the v-projection moving dim is
                # 256 (fp32r matmuls with N < 256 run at 1/4 rate)
                wq_sb = p1.tile([128, 6, 640], F32R, tag="wq")
                bias_qk = p1.tile([128, 3], F32, tag="bqk")
                bias_v = p1.tile([128, 192], F32, tag="bv")
                bias_v_row = p1.tile([1, 192], F32, tag="bvr")

                nc.vector.memset(wq_sb[:, :, 576:640].bitcast(F32), 0.0)
                for ci in range(6):
                    nc.sync.dma_start(
                        wq_sb[:, ci, 0:576],
                        wqkv_d[ci * 128 : (ci + 1) * 128, :].bitcast(F32R),
                    )
                for m in range(3):
                    nc.sync.dma_start(
                        bias_qk[:, m : m + 1],
                        bqkv_d[m * 128 : (m + 1) * 128].rearrange("(p b) -> p b", b=1),
                    )
                nc.sync.dma_start(
                    bias_v_row[0:1, :],
                    bqkv_d[384:576].rearrange("(b f) -> b f", b=1),
                )
                nc.gpsimd.partition_broadcast(bias_v[:], bias_v_row[0:1, :])

                qk_dest = [A, B_, Cc]
                for ts in range(T // 512):
                    xts = xpool.tile([128, 6, 512], F32R, tag="xts")
                    nc.sync.dma_start(
                        xts[:],
                        xt_d[:, ts * 512 : (ts + 1) * 512]
                        .rearrange("(ci p) n -> p ci n", p=128)
                        .bitcast(F32R),
                    )
                    col0 = ts * 512
                    # q/k rows (transposed layout): psum [qkv-rows, tokens]
                    for m in range(3):
                        psq = pps.tile([128, 512], F32, tag="psq")
                        for ci in range(6):
                            nc.tensor.matmul(
                                psq[:],
                                wq_sb[:, ci, m * 128 : (m + 1) * 128],
                                xts[:, ci, :],
                                start=(ci == 0),
                                stop=(ci == 5),
                            )
                        nc.vector.tensor_scalar_add(
                            out=qk_dest[m][:, col0 : col0 + 512],
                            in0=psq[:],
                            scalar1=bias_qk[:, m : m + 1],
                        )
                    # v in [token, d] layout: psum [tokens, 3*64 (+64 pad)]
                    for tb in range(4):
                        psv = vps.tile([128, 256], F32, tag="psv")
                        for ci in range(6):
                            nc.tensor.matmul(
                                psv[:],
                                xts[:, ci, tb * 128 : (tb + 1) * 128],
                                wq_sb[:, ci, 384:640],
                                start=(ci == 0),
                                stop=(ci == 5),
                            )
                        kb = ts * 4 + tb
                        nc.vector.tensor_tensor(
                            out=v_st[:, kb, :, 0:DH],
                            in0=psv[:, 0:192].rearrange("p (h d) -> p h d", h=HPC),
                            in1=bias_v[:].rearrange("p (h d) -> p h d", h=HPC),
                            op=mybir.AluOpType.add,
                        )

            # q2 lives at partitions 0-63 of Cc but k2 at 64-127; matmul needs
            # equal base partitions. Mirror both halves into D so head 2 can
            # alternate between PE tiles T0 (partitions 0-63) and T8 (64-127).
            nc.sync.dma_start(D[64:128, :], Cc[0:64, :])  # upper = q2
            nc.sync.dma_start(D[0:64, :], Cc[64:128, :])  # lower = k2

            if debug:
                nc.sync.dma_start(dbg["dbg_A"], A[:].bitcast(F32))
                nc.sync.dma_start(dbg["dbg_B"], B_[:].bitcast(F32))
                nc.sync.dma_start(dbg["dbg_C"], Cc[:].bitcast(F32))
                nc.sync.dma_start(dbg["dbg_D"], D[:].bitcast(F32))
                nc.sync.dma_start(
                    dbg["dbg_v"],
                    v_st[:].rearrange("p a b c -> p (a b c)").bitcast(F32),
                )

            # ---------------- Phase 2: attention + out-projection ----------------
            with (
                tc.tile_pool(name="p2", bufs=1) as p2,
                tc.tile_pool(name="stps", bufs=cfg["st_bufs"], space="PSUM") as stps,
                tc.tile_pool(name="avps", bufs=cfg["av_bufs"], space="PSUM") as avps,
                tc.tile_pool(name="ptp", bufs=cfg["pt_bufs"]) as ptp,
                tc.tile_pool(name="nrm", bufs=cfg["nrm_bufs"]) as nrm,
                tc.tile_pool(name="ysb", bufs=2) as ypool,
            ):
                at01 = p2.tile([128, T], F32R, tag="at01")  # [h0 d | h1 d] x q
                at2 = p2.tile([64, T], F32R, tag="at2")

                def q_ap(h, kb):
                    if h == 0:
                        return A[0:64, :]
                    if h == 1:
                        return A[64:128, :]
                    return Cc[0:64, :] if kb % 2 == 0 else D[64:128, :]

                def k_ap(h, kb):
                    if h == 0:
                        return B_[0:64, :]
                    if h == 1:
                        return B_[64:128, :]
                    return D[0:64, :] if kb % 2 == 0 else Cc[64:128, :]

                CH = cfg["st_chunk"]

                def attn_round(qs, h, kb, av):
                    """One (q-super, head, k-block) round: s^T matmuls, exp, av."""
                    q0 = qs * SUP
                    nkb = (qs + 1) * (SUP // KB)
                    last_r0 = qs * 8 + 512 // KB - 1  # last kb touching cols [0,512)
                    t = kb - qs * (SUP // KB)  # >= 0 on the diagonal
                    ext0 = max(t, 0) * KB
                    pt = ptp.tile([128, SUP], F32R, tag="pt")
                    dbg_st_tiles = []
                    for ch0 in range(0, SUP, CH):
                        ch1 = ch0 + CH
                        lo = max(ch0, ext0)
                        if lo >= ch1:
                            continue
                        st = stps.tile([128, CH], F32, tag="st")
                        if debug:
                            dbg_st_tiles.append((st, ch0))
                        c = lo
                        while c < ch1:
                            ce = min((c // 512 + 1) * 512, ch1)
                            nc.tensor.matmul(
                                st[:, c - ch0 : ce - ch0],
                                k_ap(h, kb)[:, kb * KB : (kb + 1) * KB],
                                q_ap(h, kb)[:, q0 + c : q0 + ce],
                                start=True,
                                stop=True,
                            )
                            c = ce
                        if t >= 0 and qs >= 1 and kb % 6 != 0:
                            # diagonal block: Schraudolph exp on DVE (int32 bit
                            # trick) into a scratch tile, then causal-mask
                            # multiply into pt — the f32r write the BIR
                            # verifier wants. (qs=0 stays on ACT: those rows'
                            # softmax is entirely diagonal-tile mass, so
                            # Schraudolph's ~3% error would survive.)
                            sch = ptp.tile([128, CH], I32, tag="sch")
                            nc.vector.tensor_scalar(
                                out=sch[:, lo - ch0 : CH],
                                in0=st[:, lo - ch0 : CH],
                                scalar1=float(SCH_A * SCALE),
                                scalar2=float(SCH_B),
                                op0=mybir.AluOpType.mult,
                                op1=mybir.AluOpType.add,
                            )
                            nc.vector.tensor_tensor(
                                out=pt[:, lo:ch1],
                                in0=sch[:, lo - ch0 : CH].bitcast(F32),
                                in1=mask01[:, lo - ext0 : ch1 - ext0],
                                op=mybir.AluOpType.mult,
                            )
                        else:
                            if t >= 0 and ext0 >= ch0:
                                nc.vector.tensor_tensor(
                                    out=st[:, ext0 - ch0 : ext0 - ch0 + KB],
                                    in0=st[:, ext0 - ch0 : ext0 - ch0 + KB],
                                    in1=maskadd[:],
                                    op=mybir.AluOpType.add,
                                )
                            nc.scalar.activation(
                                out=pt[:, lo:ch1],
                                in_=st[:, lo - ch0 : CH],
                                func=mybir.ActivationFunctionType.Exp,
                                bias=0.0,
                                scale=SCALE,
                            )
                    c = ext0
                    while c < SUP:
                        ce = min((c // 512 + 1) * 512, SUP)
                        stop_kb = last_r0 if ce <= 512 else nkb - 1
                        nc.tensor.matmul(
                            av[:, c:ce],
                            v_st[:, kb, h, :],
                            pt[:, c:ce],
                            start=(kb == 0),
                            stop=(kb == stop_kb),
                        )
                        c = ce
                    if debug and qs == 1 and h == 0 and kb == 0:
                        for st, ch0 in dbg_st_tiles:
                            stg = ptp.tile([128, CH], F32, tag="dbgstage")
                            nc.vector.tensor_copy(stg[:], st[:])
                            nc.sync.dma_start(dbg["dbg_st"][:, ch0 : ch0 + CH], stg[:])
                        nc.sync.dma_start(dbg["dbg_pt"], pt[:].bitcast(F32))

                def norm_head(qs, h, av):
                    """rows 0-63 of av divided by row 64, into attnT storage.
                    reciprocal_approx_fast misreads PSUM at partition offset
                    64, so evacuate av to SBUF and DMA-shift the denominator
                    row to partition 0 first."""
                    q0 = qs * SUP
                    stg = nrm.tile([65, SUP], F32, tag="avstage")
                    nc.vector.tensor_copy(stg[:], av[:])
                    if debug and qs == 1 and h == 0:
                        nc.sync.dma_start(dbg["dbg_av"], stg[:])
                    l0 = nrm.tile([1, SUP], F32, tag="l0")
                    nc.sync.dma_start(l0[0:1, :], stg[64:65, :])
                    rec = nrm.tile([1, SUP], F32, tag="rec")
                    nc.vector.reciprocal_approx_fast(out=rec[0:1, :], in_=l0[0:1, :])
                    recb = nrm.tile([64, SUP], F32, tag="recb")
                    nc.gpsimd.partition_broadcast(recb[:], rec[0:1, :])
                    if h == 0:
                        dest = at01[0:64, q0 : q0 + SUP]
                    elif h == 2:
                        dest = at2[:, q0 : q0 + SUP]
                    else:
                        # h1 rows belong at partitions 64-127 of at01; DVE
                        # can't shift partitions, so stage + DMA.
                        h1s = nrm.tile([64, SUP], F32R, tag="h1stage")
                        dest = h1s[:]
                    nc.vector.tensor_tensor(
                        out=dest, in0=stg[0:64, :], in1=recb[:], op=mybir.AluOpType.mult
                    )
                    if h == 1:
                        nc.sync.dma_start(at01[64:128, q0 : q0 + SUP], h1s[:])
                    if debug and qs == 1 and h == 0:
                        nc.sync.dma_start(dbg["dbg_rec"], rec[0:1, :])

                def out_proj(qs):
                    q0 = qs * SUP
                    for tb in range(SUP // 128):
                        tcol = q0 + tb * 128
                        yps = stps.tile([128, SUP], F32, tag="st")
                        for rs, re in ((0, 512), (512, C)):
                            nc.tensor.matmul(
                                yps[:, rs:re],
                                at01[:, tcol : tcol + 128],
                                wo01[:, rs:re],
                                start=True,
                                stop=False,
                            )
                            nc.tensor.matmul(
                                yps[:, rs:re],
                                at2[:, tcol : tcol + 128],
                                wo2[:, rs:re],
                                start=False,
                                stop=True,
                            )
                        y_sb = ypool.tile([128, C], F32, tag="ysb")
                        if cfg["yevac"] == "act":
                            nc.scalar.copy(y_sb[:], yps[:, 0:C])
                        else:
                            nc.vector.tensor_copy(y_sb[:], yps[:, 0:C])
                        nc.sync.dma_start(y_d[tcol : tcol + 128, :], y_sb[:])

                # software pipelining: emit each super's out-projection after
                # the next super's attention rounds so the PE has ready work
                # while the last norm chain (copy->DMA->recip->bcast->mul)
                # drains.
                for qs in range(NSUP):
                    q0 = qs * SUP
                    nkb = (qs + 1) * (SUP // KB)
                    for h in range(HPC):
                        av = avps.tile([65, SUP], F32, tag="av")
                        for kb in range(nkb):
                            attn_round(qs, h, kb, av)
                        norm_head(qs, h, av)
                    if qs >= 1:
                        out_proj(qs - 1)
                out_proj(NSUP - 1)
                if debug:
                    nc.sync.dma_start(dbg["dbg_at0"][0:64, 0:T], at2[:].bitcast(F32))
                    nc.sync.dma_start(
                        dbg["dbg_at0"][0:64, T : 2 * T], at01[0:64, :].bitcast(F32)
                    )
                    nc.sync.dma_start(
                        dbg["dbg_at0"][0:64, 2 * T : 3 * T],
                        at01[64:128, :].bitcast(F32),
                    )
            rep_ctx.__exit__(None, None, None)

    nc.compile()
    return nc


def shard_inputs(x, W_qkv, b_qkv, W_out, b_out):
    """Build the per-core input maps (host-side sharding)."""
    x = np.asarray(x, dtype=np.float32)
    W_qkv = np.asarray(W_qkv, dtype=np.float32)
    b_qkv = np.asarray(b_qkv, dtype=np.float32)
    W_out = np.asarray(W_out, dtype=np.float32)
    in_maps = []
    for c in range(NCORES):
        b = c // 4
        hh = (c % 4) * HPC
        h0, h1, h2 = hh, hh + 1, hh + 2

        def qcols(h):
            return list(range(h * DH, (h + 1) * DH))

        def kcols(h):
            return list(range(C + h * DH, C + (h + 1) * DH))

        def vcols(h):
            return list(range(2 * C + h * DH, 2 * C + (h + 1) * DH))

        perm = (
            qcols(h0) + qcols(h1) + kcols(h0) + kcols(h1) + qcols(h2) + kcols(h2)
            + vcols(h0) + vcols(h1) + vcols(h2)
        )
        in_maps.append(
            {
                "xt": np.ascontiguousarray(x[b].T),
                "wqkv": np.ascontiguousarray(W_qkv[:, perm]),
                "bqkv": np.ascontiguousarray(b_qkv[perm]),
                "wout": np.ascontiguousarray(W_out[hh * DH : (hh + HPC) * DH, :]),
            }
        )
    return in_maps


def kernel(x, W_qkv, b_qkv, W_out, b_out):
    global _PROG, LAST_RESULT
    if _PROG is None:
        _PROG = build_program()
    nc = _PROG
    in_maps = shard_inputs(x, W_qkv, b_qkv, W_out, b_out)
    res = run_bass_kernel_spmd(nc, in_maps, list(range(NCORES)), trace=TRACE)
    LAST_RESULT = res
    b_out = np.asarray(b_out, dtype=np.float32)
    y = np.zeros((2, T, C), dtype=np.float32)
    for c in range(NCORES):
        y[c // 4] += res.results[c]["y"]
    y += b_out[None, None, :]
    return y
